# revision 1
# baseline (speedup 1.0000x reference)
"""BioRNN Trainium2 kernel (dev module).

Per-core math (batch-sharded 8-way, B=8 per core):
    z'_t = 0.2*(x_t @ w_in + noise_t + b_rec)        (precomputed, fp16, T layout)
    h_t  = 0.8*h_{t-1} + relu(z'_t + h_{t-1} @ W')   W' = 0.2*w_eff, fp16
Output h_t (B, T, 512) f32.

T layout: partition dim = n_rec slice (4 chunks of 128); free dim packs
(t, m, b): per-step supertile of 32 cols = 4 m-chunks x 8 batch.
  zbuf  sbuf fp16 (128, T*32)   col = t*32 + m*8 + b
  h16   sbuf fp16 (128, U*32)   ring of U steps, same col layout
  w16   sbuf fp16 (128, 4*512)  [p, k*512 + m*128 + c] = W'[k*128+p, m*128+c]
  xT16  sbuf fp16 (128, T*8)    col = t*8 + b   (n_in on partitions)

Recurrence step: 16 matmuls (lhsT = w16 tile (128,128), rhs = h16 slice
(128,8)) accumulate into psum (128, 2048) f32, bank m cols [512m, 512m+8).
Then per half (m pair): TT-add psum+zbuf -> r32, relu in place, STT
h_new = 0.8*h_old + r32 -> h16 ring (fp16).

Output path: PE-transpose h16 (128 r, blk t) -> psum fp16 (t, r), DVE cast
to f32 staging, DMA to out[b, t, r].
"""

import numpy as np
from contextlib import ExitStack

import concourse.bass as bass
import concourse.mybir as mybir
import concourse.tile as tile
from concourse import bacc
from concourse import dve_ops
from concourse.dve_spec import Spec, Src0, Src1, C0, relu as _dve_relu_expr, lower
from concourse.dve_uop import DveOpSpec
from concourse.masks import make_identity


def _register_relu_add_sc():
    """Register fused out = relu((in0 + in1) * s0) custom DVE op (idempotent)."""
    name = "RELU_ADD_SC_BIO"
    for o in dve_ops.OPS:
        if o.name == name:
            return o
    opcode = max(dve_ops._SUB_OPCODE_FOR_NAME.values()) + 1
    assert opcode < 0x20
    dve_ops._SUB_OPCODE_FOR_NAME[name] = opcode

    def _ref(in0, in1, c0, c1, c2):
        a = in0.astype(np.float32).reshape(in0.shape[0], -1)
        b = in1.astype(np.float32).reshape(in1.shape[0], -1)
        s = np.maximum(np.nan_to_num((a + b) * c0, nan=0.0, posinf=np.inf,
                                     neginf=-np.inf), 0)
        return s.reshape(in0.shape)

    spec = Spec(body=_dve_relu_expr((Src0 + Src1) * C0), reference=_ref)
    shas = {}
    for ver in ("v3", "v4"):
        s = DveOpSpec(name=name, opcode=opcode, uops=lower(spec, ver=ver),
                      rd1_en=True)
        shas[ver] = s.sha(ver)
    op = dve_ops.DveOp(name, spec, subdim=False, uops_sha=shas)
    dve_ops.OPS.append(op)
    dve_ops.CUSTOM_DVE_SPECS[name] = spec
    return op


RELU_ADD_SC = _register_relu_add_sc()

F32 = mybir.dt.float32
F16 = mybir.dt.float16
AOP = mybir.AluOpType

B = 8            # batch per core
R = 512          # n_rec
NIN = 128        # n_in
RC = 4           # r chunks (m and k)
SUP = RC * B     # 32 cols per step supertile
N_CORES = 8
ALPHA = 0.2
LEAK = 1.0 - ALPHA


def build_nc(T=1000, U=256, use_bacc=True):
    """Build the per-core Bass program. U = h-ring steps."""
    OBLK = 128  # output transpose block (steps)
    assert U % (2 * OBLK) == 0
    nc = bacc.Bacc() if use_bacc else bass.Bass()

    x_d = nc.dram_tensor("x_c", [B, T, NIN], F32, kind="ExternalInput").ap()
    n_d = nc.dram_tensor("noise_c", [B, T, R], F32, kind="ExternalInput").ap()
    w_d = nc.dram_tensor("w16", [R, R], F16, kind="ExternalInput").ap()
    wi_d = nc.dram_tensor("win16", [NIN, R], F16, kind="ExternalInput").ap()
    b_d = nc.dram_tensor("b32", [R], F32, kind="ExternalInput").ap()
    o_d = nc.dram_tensor("out_c", [B, T, R], F32, kind="ExternalOutput").ap()

    ZB = 64  # zmm steps per matmul (64 steps = 512 moving cols)

    with tile.TileContext(nc) as tc, ExitStack() as ctx:
        const = ctx.enter_context(tc.tile_pool(name="const", bufs=1))
        big = ctx.enter_context(tc.tile_pool(name="big", bufs=1))
        dram = ctx.enter_context(tc.tile_pool(name="dram", bufs=1, space="DRAM"))

        # ---- constants ----
        ident16 = const.tile([128, 128], F16)
        make_identity(nc, ident16[:, :])

        w16 = const.tile([128, RC * R], F16)
        nc.sync.dma_start(
            out=w16[:, :].rearrange("p (k m) -> p k m", m=R),
            in_=w_d.rearrange("(k p) m -> p k m", p=128),
        )
        win16 = const.tile([128, R], F16)
        nc.sync.dma_start(out=win16[:, :], in_=wi_d)
        b32 = const.tile([128, RC], F32)
        nc.sync.dma_start(out=b32[:, :], in_=b_d.rearrange("(m p) -> p m", p=128))

        # ---- big persistent buffers ----
        # zbuf m-major planes: col = m*(T*B) + t*B + b
        zbuf = big.tile([128, RC * T * B], F16)
        xT16 = big.tile([128, T * B], F16)
        h16 = big.tile([128, U * SUP], F16)
        nc.vector.memset(h16[:, (U - 1) * SUP:U * SUP], 0.0)

        zv = zbuf[:, :].rearrange("p (m t b) -> p m t b", t=T, b=B)
        hv = h16[:, :].rearrange("p (t m b) -> p t m b", m=RC, b=B)

        # ---- prepass: DMA cast+reorder to (t,b,r) scratch, then xbar ----
        nscr = dram.tile([T * B, R], F16)
        xscr = dram.tile([T * B, NIN], F16)
        nv = nscr[:, :].rearrange("(t b) r -> t b r", b=B)
        xv_s = xscr[:, :].rearrange("(t b) r -> t b r", b=B)
        ps_z = ctx.enter_context(tc.tile_pool(name="psz", bufs=2, space="PSUM"))
        PIECES = [(0, min(128, T))]
        if T > 128:
            PIECES.append((128, min(448, T)))
        if T > 448:
            PIECES.append((448, T))
        for (t0, t1) in PIECES:
            for b in range(B):
                nc.gpsimd.dma_start(out=nv[t0:t1, b, :], in_=n_d[b, t0:t1, :])
                nc.gpsimd.dma_start(out=xv_s[t0:t1, b, :], in_=x_d[b, t0:t1, :])
        for (t0, t1) in PIECES:
            for m in range(RC):
                nc.sync.dma_start(
                    out=zv[:, m, t0:t1, :].rearrange("p t b -> p (t b)"),
                    in_=nscr[t0 * B:t1 * B, m * 128:(m + 1) * 128],
                    transpose=True,
                )
            nc.sync.dma_start(out=xT16[:, t0 * B:t1 * B],
                              in_=xscr[t0 * B:t1 * B, :], transpose=True)

        def emit_prepass_zmm(p0, p1):
            # zbuf += x @ w_in + b_rec (0.2 applied in RELU_ADD_SC)
            for z0 in range(p0, p1, ZB):
                nt = min(ZB, p1 - z0)
                for m in range(RC):
                    zps = ps_z.tile([128, ZB * B], F32, tag="zps")
                    nc.tensor.matmul(
                        zps[:, :nt * B],
                        lhsT=win16[:, m * 128:(m + 1) * 128],
                        rhs=xT16[:, z0 * B:(z0 + nt) * B],
                        start=True, stop=True,
                    )
                    zsl = zv[:, m, z0:z0 + nt, :]
                    nc.vector.scalar_tensor_tensor(
                        out=zsl,
                        in0=zps[:, :nt * B].rearrange("p (t b) -> p t b", b=B),
                        scalar=b32[:, m:m + 1], in1=zsl,
                        op0=AOP.add, op1=AOP.add,
                    )

        # ---- recurrence + interleaved output drain ----
        # psum-resident recurrence: p_{t+1} = 0.8*p_t + r_t @ W
        #   r_t = relu((p_t + z_t) * 0.2)     (fp16, feeds next burst)
        #   h_t = 0.8*h_{t-1} + r_t           (fp16, output only)
        # Burst order per step: A=[k01 all m] C1=[m01 k23] I01 C2=[m23 k23] I23
        # so RA_a (banks m01) can run while PE does C2/I23.
        with tc.tile_pool(name="rp", bufs=2) as rp, \
             tc.tile_pool(name="sp", bufs=2) as sp, \
             tc.tile_pool(name="ostg", bufs=3) as ostg, \
             tc.tile_pool(name="psA0", bufs=1, space="PSUM") as ps_a0, \
             tc.tile_pool(name="psA1", bufs=1, space="PSUM") as ps_a1, \
             tc.tile_pool(name="psC0", bufs=1, space="PSUM") as ps_c0, \
             tc.tile_pool(name="psC1", bufs=1, space="PSUM") as ps_c1, \
             tc.tile_pool(name="psot", bufs=2, space="PSUM") as ps_ot:
            # one bank per half; two m-chunks at 128-col offsets; ping-pong
            # across step parity so a new burst never WARs pending readers.
            psAs = [ps_a0.tile([128, 512], F32, name="psa0", tag="psa0"),
                    ps_a1.tile([128, 512], F32, name="psa1", tag="psa1")]
            psCs = [ps_c0.tile([128, 512], F32, name="psc0", tag="psc0"),
                    ps_c1.tile([128, 512], F32, name="psc1", tag="psc1")]
            pvAs = [p[:, :].rearrange("p (m c) -> p m c", c=128) for p in psAs]
            pvCs = [p[:, :].rearrange("p (m c) -> p m c", c=128) for p in psCs]

            zero16 = const.tile([128, B], F16)
            nc.vector.memset(zero16[:, :], 0.0)

            def ps_of(m, par):
                ps = psAs[par] if m < 2 else psCs[par]
                return ps, (m % 2) * 128

            pending = []

            def emit_out_unit(u):
                blk_t0, nt, b, m = u
                rt0 = blk_t0 % U
                tp = ps_ot.tile([128, OBLK], F16, tag="otp")
                tr = nc.tensor.transpose(tp[:nt, :128],
                                         hv[:, rt0:rt0 + nt, m, b],
                                         ident16[:, :])
                st = ostg.tile([128, 128], F32, tag="ost")
                nc.scalar.copy(out=st[:nt, :], in_=tp[:nt, :128])
                nc.sync.dma_start(
                    out=o_d[b, blk_t0:blk_t0 + nt, m * 128:(m + 1) * 128],
                    in_=st[:nt, :],
                )
                return tr

            # prime p_0 = 0 (parity 0 banks; m%2==0 start clears the bank,
            # m%2==1 then overwrites via cleared has_written bits)
            for m in range(RC):
                ps, off = ps_of(m, 0)
                nc.tensor.matmul(ps[:, off:off + B], lhsT=w16[:, 0:128],
                                 rhs=zero16[:, :], start=(m % 2 == 0),
                                 stop=True, skip_group_check=True)

            emit_prepass_zmm(*PIECES[0])
            prev_ra = prev_rb = prev_sa = prev_sb = None
            for t in range(T):
                for pi in range(1, len(PIECES)):
                    if t == PIECES[pi][0] - 64:
                        emit_prepass_zmm(*PIECES[pi])
                rd = ((t - 1) % U) * SUP
                wr = (t % U) * SUP
                r16a = rp.tile([128, 16], F16, tag="r16a")
                r16b = rp.tile([128, 16], F16, tag="r16b")
                s16a = sp.tile([128, 16], F16, tag="s16a")
                s16b = sp.tile([128, 16], F16, tag="s16b")
                par = t % 2
                if t > 0:
                    def kmm(m, k, start=False, stop=False):
                        ps, off = ps_of(m, par)
                        src = prev_ra if k < 2 else prev_rb
                        return nc.tensor.matmul(
                            ps[:, off:off + B],
                            lhsT=w16[:, k * R + m * 128:k * R + (m + 1) * 128],
                            rhs=src[:, (k % 2) * B:(k % 2 + 1) * B],
                            start=start, stop=stop, skip_group_check=True,
                        )

                    def imm(m):
                        ps, off = ps_of(m, par)
                        src = prev_sa if m < 2 else prev_sb
                        return nc.tensor.matmul(
                            ps[:, off:off + B], lhsT=ident16[:, :],
                            rhs=src[:, (m % 2) * B:(m % 2 + 1) * B],
                            start=False, stop=True, skip_group_check=True,
                        )

                    for k in (0, 1):              # A: k01, all m
                        for m in range(RC):
                            kmm(m, k, start=(k == 0 and m % 2 == 0))
                    for m in (0, 1):              # C1: m01 k23
                        kmm(m, 2)
                        kmm(m, 3)
                    imm(0)                        # I01
                    i01_last = imm(1)
                    first_c2 = kmm(2, 2)          # C2: m23 k23
                    tile.add_dep_helper(
                        first_c2.ins, i01_last.ins, sync=False,
                        reason="keep I01 before C2 so RA_a unblocks early")
                    kmm(2, 3)
                    kmm(3, 2)
                    kmm(3, 3)
                    imm(2)                        # I23
                    last_mm = imm(3)

                # RA halves (DVE) + 0.8*p copies (ACT)
                nc.vector._custom_dve(
                    RELU_ADD_SC,
                    out=r16a[:, :].rearrange("p (m c) -> p m c", c=B),
                    in0=pvAs[par][:, 0:2, 0:B], in1=zv[:, 0:2, t, :],
                    s0=ALPHA)
                nc.scalar.mul(
                    out=s16a[:, :].rearrange("p (m c) -> p m c", c=B),
                    in_=pvAs[par][:, 0:2, 0:B], mul=LEAK)
                nc.vector._custom_dve(
                    RELU_ADD_SC,
                    out=r16b[:, :].rearrange("p (m c) -> p m c", c=B),
                    in0=pvCs[par][:, 0:2, 0:B], in1=zv[:, 2:4, t, :],
                    s0=ALPHA)
                nc.scalar.mul(
                    out=s16b[:, :].rearrange("p (m c) -> p m c", c=B),
                    in_=pvCs[par][:, 0:2, 0:B], mul=LEAK)
                # h output (off critical path)
                nc.vector.scalar_tensor_tensor(
                    out=h16[:, wr:wr + 16], in0=h16[:, rd:rd + 16],
                    scalar=LEAK, in1=r16a[:, :],
                    op0=AOP.mult, op1=AOP.add,
                )
                nc.vector.scalar_tensor_tensor(
                    out=h16[:, wr + 16:wr + SUP], in0=h16[:, rd + 16:rd + SUP],
                    scalar=LEAK, in1=r16b[:, :],
                    op0=AOP.mult, op1=AOP.add,
                )
                prev_ra, prev_rb = r16a, r16b
                prev_sa, prev_sb = s16a, s16b
                if (t + 1) % OBLK == 0 or t == T - 1:
                    blk_t0 = (t // OBLK) * OBLK
                    for b in range(B):
                        for m in range(RC):
                            pending.append((blk_t0, t + 1 - blk_t0, b, m))
                if pending and t >= OBLK:
                    emit_out_unit(pending.pop(0))
            while pending:
                emit_out_unit(pending.pop(0))

    if use_bacc:
        nc.compile()
    return nc


def host_prep(x, w_in, w_rec, b_rec, ei_mask, autapse_mask, noise):
    """Host-side weight prep + batch shard. Returns list of per-core in_maps."""
    ei = np.diagonal(np.asarray(ei_mask)).astype(np.float32)
    w_eff = ei[:, None] * (np.asarray(w_rec) * np.asarray(autapse_mask))
    w16 = w_eff.astype(np.float16)
    win16 = np.asarray(w_in).astype(np.float16)
    b32 = np.asarray(b_rec).astype(np.float32)
    x = np.asarray(x, dtype=np.float32)
    noise = np.asarray(noise, dtype=np.float32)
    bs = x.shape[0] // N_CORES
    in_maps = []
    for c in range(N_CORES):
        in_maps.append({
            "x_c": np.ascontiguousarray(x[c * bs:(c + 1) * bs]),
            "noise_c": np.ascontiguousarray(noise[c * bs:(c + 1) * bs]),
            "w16": w16,
            "win16": win16,
            "b32": b32,
        })
    return in_maps, w_eff.astype(np.float32)


def reference_np(x, w_in, b_rec, w_eff, noise, T=None):
    """Numpy reference for dev checks (f32)."""
    x = np.asarray(x, np.float32)
    if T is None:
        T = x.shape[1]
    z = np.einsum("bti,ir->btr", x[:, :T], np.asarray(w_in)) \
        + np.asarray(noise)[:, :T] + np.asarray(b_rec)
    h = np.zeros((x.shape[0], w_eff.shape[0]), np.float32)
    outs = []
    for t in range(T):
        pre = z[:, t] + h @ w_eff
        h = LEAK * h + ALPHA * np.maximum(pre, 0.0)
        outs.append(h.copy())
    return np.stack(outs, axis=1)


# ---------------------------------------------------------------------------
# harness entry point
# ---------------------------------------------------------------------------
_NC_CACHE = {}


def kernel(x, w_in, w_rec, b_rec, ei_mask, autapse_mask, noise):
    from concourse.bass_utils import run_bass_kernel_spmd

    x = np.asarray(x)
    T = x.shape[1]
    in_maps, _ = host_prep(x, w_in, w_rec, b_rec, ei_mask, autapse_mask, noise)
    if T not in _NC_CACHE:
        _NC_CACHE[T] = build_nc(T=T)
    nc = _NC_CACHE[T]
    res = run_bass_kernel_spmd(nc, in_maps, core_ids=list(range(N_CORES)))
    out = np.concatenate([r["out_c"] for r in res.results], axis=0)
    return out.astype(np.float32)



# revision 2
# speedup vs baseline: 1.3761x; 1.3761x over previous
"""BioRNN Trainium2 kernel v2.

Per-core math (batch-sharded 8-way, B=8 per core):
    z_t = x_t @ w_in + noise_t + b_rec          (precomputed, fp16)
    p_t = z_t + h_{t-1} @ W                     (psum, rebuilt each step)
    h_t = 0.8*h_{t-1} + relu(0.2 * p_t)         (fused DVE op, fp16)

Layouts (partition dim = r-chunk of 128; 4 chunks m=0..3):
  zbuf sbuf fp16 (128, 4*T*8)    col = m*(T*8) + t*8 + b   (m-major planes)
  h16  sbuf fp16 (128, (T+1)*32) col = s*32 + m*8 + b, slot s = h_{s-1}
  w16  sbuf fp16 (128, 4*512)    [p, k*512 + m*128 + c] = W[k*128+p, m*128+c]
  xT16 sbuf fp16 (128, T*8)      col = t*8 + b  (n_in on partitions)

Per step (parity par = t%2, two psum banks per parity):
  phase A (bank A[par], m01): imm01 (identity x z, start=True clears bank),
    then kmm (k,m) for k=0..3 x m=0,1;   DVE01: h_new[m01] from psum+h_old
  phase B (bank C[par], m23): same for m=2,3;  DVE23.
kmm(k,m): lhsT = w16 tile (128,128), rhs = h16 slot t cols k*8..k*8+8.

Output: h16 slabs DMA'd raw to DRAM fp16; host reshapes to (b, t, r) f32.
"""

import numpy as np
from contextlib import ExitStack

import concourse.bass as bass
import concourse.mybir as mybir
import concourse.tile as tile
from concourse import bacc
from concourse import dve_ops
from concourse.dve_spec import Spec, Src0, Src1, C0, C1, relu as _relu, lower
from concourse.dve_uop import DveOpSpec
from concourse.masks import make_identity


def _register_leaky_relu_acc():
    """Register fused out = relu(in0*s0) + in1*s1 custom DVE op (idempotent)."""
    name = "LEAKY_RELU_ACC_BIO"
    for o in dve_ops.OPS:
        if o.name == name:
            return o
    opcode = max(dve_ops._SUB_OPCODE_FOR_NAME.values()) + 1
    assert opcode < 0x20
    dve_ops._SUB_OPCODE_FOR_NAME[name] = opcode

    def _ref(in0, in1, c0, c1, c2):
        a = in0.astype(np.float32).reshape(in0.shape[0], -1)
        b = in1.astype(np.float32).reshape(in1.shape[0], -1)
        s = np.maximum(np.nan_to_num(a * c0, nan=0.0, posinf=np.inf,
                                     neginf=-np.inf), 0) + b * c1
        return s.reshape(in0.shape)

    spec = Spec(body=_relu(Src0 * C0) + Src1 * C1, reference=_ref)
    shas = {}
    for ver in ("v3", "v4"):
        s = DveOpSpec(name=name, opcode=opcode, uops=lower(spec, ver=ver),
                      rd1_en=True)
        shas[ver] = s.sha(ver)
    op = dve_ops.DveOp(name, spec, subdim=False, uops_sha=shas)
    dve_ops.OPS.append(op)
    dve_ops.CUSTOM_DVE_SPECS[name] = spec
    return op


LEAKY_RELU_ACC = _register_leaky_relu_acc()

F32 = mybir.dt.float32
F16 = mybir.dt.float16
AOP = mybir.AluOpType

B = 8            # batch per core
R = 512          # n_rec
NIN = 128        # n_in
RC = 4           # r chunks
SUP = RC * B     # 32 cols per step supertile
N_CORES = 8
ALPHA = 0.2
LEAK = 1.0 - ALPHA


def build_nc(T=1000, use_bacc=True):
    nc = bacc.Bacc() if use_bacc else bass.Bass()

    z_d = nc.dram_tensor("nz16", [128, RC * T * B], F16, kind="ExternalInput").ap()
    x_d = nc.dram_tensor("xT16", [128, T * B], F16, kind="ExternalInput").ap()
    w_d = nc.dram_tensor("w16", [128, RC * R], F16, kind="ExternalInput").ap()
    wi_d = nc.dram_tensor("win16", [NIN, R], F16, kind="ExternalInput").ap()
    o_d = nc.dram_tensor("out16", [128, T * SUP], F16, kind="ExternalOutput").ap()

    ZB = 64  # zmm steps per prepass matmul (64 steps = 512 moving cols)

    with tile.TileContext(nc) as tc, ExitStack() as ctx:
        const = ctx.enter_context(tc.tile_pool(name="const", bufs=1))
        big = ctx.enter_context(tc.tile_pool(name="big", bufs=1))

        ident16 = const.tile([128, 128], F16)
        make_identity(nc, ident16[:, :])

        w16 = const.tile([128, RC * R], F16)
        nc.sync.dma_start(out=w16[:, :], in_=w_d)
        win16 = const.tile([128, R], F16)
        nc.sync.dma_start(out=win16[:, :], in_=wi_d)

        zbuf = big.tile([128, RC * T * B], F16)
        xT16 = big.tile([128, T * B], F16)
        h16 = big.tile([128, (T + 1) * SUP], F16)
        nc.vector.memset(h16[:, 0:SUP], 0.0)

        zv = zbuf[:, :].rearrange("p (m t b) -> p m t b", t=T, b=B)
        zd_v = z_d.rearrange("p (m t b) -> p m t b", t=T, b=B)

        PIECES = [(0, min(128, T))]
        if T > 128:
            PIECES.append((128, min(448, T)))
        if T > 448:
            PIECES.append((448, T))

        # input DMA per piece (noise+b preformatted on host; x transposed)
        for (t0, t1) in PIECES:
            nc.gpsimd.dma_start(out=zv[:, :, t0:t1, :], in_=zd_v[:, :, t0:t1, :])
            nc.gpsimd.dma_start(out=xT16[:, t0 * B:t1 * B],
                                in_=x_d[:, t0 * B:t1 * B])

        ps_z = ctx.enter_context(tc.tile_pool(name="psz", bufs=2, space="PSUM"))

        def emit_prepass_zmm(p0, p1):
            # zbuf += x @ w_in  (noise + b already in zbuf from DMA)
            for z0 in range(p0, p1, ZB):
                nt = min(ZB, p1 - z0)
                for m in range(RC):
                    zps = ps_z.tile([128, ZB * B], F32, tag="zps")
                    nc.tensor.matmul(
                        zps[:, :nt * B],
                        lhsT=win16[:, m * 128:(m + 1) * 128],
                        rhs=xT16[:, z0 * B:(z0 + nt) * B],
                        start=True, stop=True,
                    )
                    zsl = zv[:, m, z0:z0 + nt, :]
                    nc.vector.scalar_tensor_tensor(
                        out=zsl,
                        in0=zps[:, :nt * B].rearrange("p (t b) -> p t b", b=B),
                        scalar=0.0, in1=zsl,
                        op0=AOP.add, op1=AOP.add,
                    )

        # ---- recurrence ----
        with tc.tile_pool(name="psA0", bufs=1, space="PSUM") as ps_a0, \
             tc.tile_pool(name="psA1", bufs=1, space="PSUM") as ps_a1, \
             tc.tile_pool(name="psC0", bufs=1, space="PSUM") as ps_c0, \
             tc.tile_pool(name="psC1", bufs=1, space="PSUM") as ps_c1:
            psAs = [ps_a0.tile([128, 512], F32, name="psa0", tag="psa0"),
                    ps_a1.tile([128, 512], F32, name="psa1", tag="psa1")]
            psCs = [ps_c0.tile([128, 512], F32, name="psc0", tag="psc0"),
                    ps_c1.tile([128, 512], F32, name="psc1", tag="psc1")]
            pvAs = [p[:, :].rearrange("p (m c) -> p m c", c=128) for p in psAs]
            pvCs = [p[:, :].rearrange("p (m c) -> p m c", c=128) for p in psCs]

            emit_prepass_zmm(*PIECES[0])

            for t in range(T):
                for pi in range(1, len(PIECES)):
                    if t == PIECES[pi][0] - 64:
                        emit_prepass_zmm(*PIECES[pi])
                par = t % 2
                rd = t * SUP
                wr = (t + 1) * SUP

                def kmm(m, k, stop=False):
                    ps = psAs[par] if m < 2 else psCs[par]
                    off = (m % 2) * 128
                    return nc.tensor.matmul(
                        ps[:, off:off + B],
                        lhsT=w16[:, k * R + m * 128:k * R + (m + 1) * 128],
                        rhs=h16[:, rd + k * B:rd + (k + 1) * B],
                        start=False, stop=stop, skip_group_check=True,
                    )

                def imm2(half):
                    # one matmul injecting z for both m-chunks of the half
                    pv = pvAs[par] if half == 0 else pvCs[par]
                    return nc.tensor.matmul(
                        pv[:, 0:2, 0:B], lhsT=ident16[:, :],
                        rhs=zv[:, 2 * half:2 * half + 2, t, :],
                        start=True, stop=False, skip_group_check=True,
                    )

                # phase A: m01
                imm2(0)
                for k in range(RC):
                    kmm(0, k, stop=(k == RC - 1))
                    kmm(1, k, stop=(k == RC - 1))
                nc.vector._custom_dve(
                    LEAKY_RELU_ACC,
                    out=h16[:, wr:wr + 16].rearrange("p (m c) -> p m c", c=B),
                    in0=pvAs[par][:, 0:2, 0:B],
                    in1=h16[:, rd:rd + 16].rearrange("p (m c) -> p m c", c=B),
                    s0=ALPHA, s1=LEAK)
                # phase B: m23
                imm2(1)
                for k in range(RC):
                    kmm(2, k, stop=(k == RC - 1))
                    kmm(3, k, stop=(k == RC - 1))
                nc.vector._custom_dve(
                    LEAKY_RELU_ACC,
                    out=h16[:, wr + 16:wr + SUP].rearrange("p (m c) -> p m c", c=B),
                    in0=pvCs[par][:, 0:2, 0:B],
                    in1=h16[:, rd + 16:rd + SUP].rearrange("p (m c) -> p m c", c=B),
                    s0=ALPHA, s1=LEAK)

                # output drain: raw fp16 slabs, host does the reshape
                if (t + 1) % 128 == 0 or t == T - 1:
                    t0 = (t // 128) * 128
                    nc.sync.dma_start(
                        out=o_d[:, t0 * SUP:(t + 1) * SUP],
                        in_=h16[:, (t0 + 1) * SUP:(t + 2) * SUP],
                    )

    if use_bacc:
        nc.compile()
    return nc


def host_prep(x, w_in, w_rec, b_rec, ei_mask, autapse_mask, noise):
    """Host-side weight prep + layout marshalling + batch shard."""
    ei = np.diagonal(np.asarray(ei_mask)).astype(np.float32)
    w_eff = ei[:, None] * (np.asarray(w_rec) * np.asarray(autapse_mask))
    # w16[p, k*512 + m*128 + c] = w_eff[k*128+p, m*128+c]
    w16 = np.ascontiguousarray(
        w_eff.reshape(RC, 128, RC, 128).transpose(1, 0, 2, 3)
        .reshape(128, RC * R)).astype(np.float16)
    win16 = np.asarray(w_in).astype(np.float16)
    x = np.asarray(x, dtype=np.float32)
    T = x.shape[1]
    nz = (np.asarray(noise, dtype=np.float32)
          + np.asarray(b_rec, dtype=np.float32)).astype(np.float16)
    x16 = x.astype(np.float16)
    bs = x.shape[0] // N_CORES
    in_maps = []
    for c in range(N_CORES):
        xc = x16[c * bs:(c + 1) * bs]                      # (B, T, NIN)
        nc_ = nz[c * bs:(c + 1) * bs]                      # (B, T, R)
        xT = np.ascontiguousarray(
            xc.transpose(2, 1, 0).reshape(128, T * B))     # [i, t*8+b]
        nzc = np.ascontiguousarray(
            nc_.reshape(B, T, RC, 128).transpose(3, 2, 1, 0)
            .reshape(128, RC * T * B))                     # [p, m, t, b]
        in_maps.append({
            "nz16": nzc,
            "xT16": xT,
            "w16": w16,
            "win16": win16,
        })
    return in_maps, w_eff.astype(np.float32)


def reference_np(x, w_in, b_rec, w_eff, noise, T=None):
    x = np.asarray(x, np.float32)
    if T is None:
        T = x.shape[1]
    z = np.einsum("bti,ir->btr", x[:, :T], np.asarray(w_in)) \
        + np.asarray(noise)[:, :T] + np.asarray(b_rec)
    h = np.zeros((x.shape[0], w_eff.shape[0]), np.float32)
    outs = []
    for t in range(T):
        pre = z[:, t] + h @ w_eff
        h = LEAK * h + ALPHA * np.maximum(pre, 0.0)
        outs.append(h.copy())
    return np.stack(outs, axis=1)


# ---------------------------------------------------------------------------
# harness entry point
# ---------------------------------------------------------------------------
_NC_CACHE = {}


def kernel(x, w_in, w_rec, b_rec, ei_mask, autapse_mask, noise):
    from concourse.bass_utils import run_bass_kernel_spmd

    x = np.asarray(x)
    T = x.shape[1]
    in_maps, _ = host_prep(x, w_in, w_rec, b_rec, ei_mask, autapse_mask, noise)
    if T not in _NC_CACHE:
        _NC_CACHE[T] = build_nc(T=T)
    nc = _NC_CACHE[T]
    res = run_bass_kernel_spmd(nc, in_maps, core_ids=list(range(N_CORES)))
    outs = []
    for r in res.results:
        a = r["out16"].reshape(128, T, RC, B)
        outs.append(np.ascontiguousarray(a.transpose(3, 1, 2, 0))
                    .reshape(B, T, R).astype(np.float32))
    return np.concatenate(outs, axis=0)


# revision 9
# speedup vs baseline: 1.4730x; 1.0704x over previous
"""BioRNN Trainium2 kernel v2.

Per-core math (batch-sharded 8-way, B=8 per core):
    z_t = x_t @ w_in + noise_t + b_rec          (precomputed, fp16)
    p_t = z_t + h_{t-1} @ W                     (psum, rebuilt each step)
    h_t = 0.8*h_{t-1} + relu(0.2 * p_t)         (fused DVE op, fp16)

Layouts (partition dim = r-chunk of 128; 4 chunks m=0..3):
  zbuf sbuf fp16 (128, 4*T*8)    col = m*(T*8) + t*8 + b   (m-major planes)
  h16  sbuf fp16 (128, (T+1)*32) col = s*32 + m*8 + b, slot s = h_{s-1}
  w16  sbuf fp16 (128, 4*512)    [p, k*512 + m*128 + c] = W[k*128+p, m*128+c]
  xT16 sbuf fp16 (128, T*8)      col = t*8 + b  (n_in on partitions)

Per step (parity par = t%2, two psum banks per parity):
  phase A (bank A[par], m01): imm01 (identity x z, start=True clears bank),
    then kmm (k,m) for k=0..3 x m=0,1;   DVE01: h_new[m01] from psum+h_old
  phase B (bank C[par], m23): same for m=2,3;  DVE23.
kmm(k,m): lhsT = w16 tile (128,128), rhs = h16 slot t cols k*8..k*8+8.

Output: h16 slabs DMA'd raw to DRAM fp16; host reshapes to (b, t, r) f32.
"""

import numpy as np
from contextlib import ExitStack

import concourse.bass as bass
import concourse.mybir as mybir
import concourse.tile as tile
from concourse import bacc
from concourse import dve_ops
from concourse.dve_spec import Spec, Src0, Src1, C0, C1, relu as _relu, lower
from concourse.dve_uop import DveOpSpec
from concourse.masks import make_identity


def _register_leaky_relu_acc():
    """Register fused out = relu(in0*s0) + in1*s1 custom DVE op (idempotent)."""
    name = "LEAKY_RELU_ACC_BIO"
    for o in dve_ops.OPS:
        if o.name == name:
            return o
    opcode = max(dve_ops._SUB_OPCODE_FOR_NAME.values()) + 1
    assert opcode < 0x20
    dve_ops._SUB_OPCODE_FOR_NAME[name] = opcode

    def _ref(in0, in1, c0, c1, c2):
        a = in0.astype(np.float32).reshape(in0.shape[0], -1)
        b = in1.astype(np.float32).reshape(in1.shape[0], -1)
        s = np.maximum(np.nan_to_num(a * c0, nan=0.0, posinf=np.inf,
                                     neginf=-np.inf), 0) + b * c1
        return s.reshape(in0.shape)

    spec = Spec(body=_relu(Src0 * C0) + Src1 * C1, reference=_ref)
    shas = {}
    for ver in ("v3", "v4"):
        s = DveOpSpec(name=name, opcode=opcode, uops=lower(spec, ver=ver),
                      rd1_en=True)
        shas[ver] = s.sha(ver)
    op = dve_ops.DveOp(name, spec, subdim=False, uops_sha=shas)
    dve_ops.OPS.append(op)
    dve_ops.CUSTOM_DVE_SPECS[name] = spec
    return op


LEAKY_RELU_ACC = _register_leaky_relu_acc()

F32 = mybir.dt.float32
F16 = mybir.dt.float16
AOP = mybir.AluOpType

B = 8            # batch per core
R = 512          # n_rec
NIN = 128        # n_in
RC = 4           # r chunks
SUP = RC * B     # 32 cols per step supertile
N_CORES = 8
ALPHA = 0.2
LEAK = 1.0 - ALPHA


def build_nc(T=1000, use_bacc=True):
    nc = bacc.Bacc() if use_bacc else bass.Bass()

    z_d = nc.dram_tensor("nz16", [128, RC * T * B], F16, kind="ExternalInput").ap()
    x_d = nc.dram_tensor("xT16", [128, T * B], F16, kind="ExternalInput").ap()
    w_d = nc.dram_tensor("w16", [128, RC * R], F16, kind="ExternalInput").ap()
    wi_d = nc.dram_tensor("win16", [NIN, R], F16, kind="ExternalInput").ap()
    o_d = nc.dram_tensor("out16", [128, T * SUP], F16, kind="ExternalOutput").ap()

    ZB = 64  # zmm steps per prepass matmul (64 steps = 512 moving cols)

    with tile.TileContext(nc) as tc, ExitStack() as ctx:
        const = ctx.enter_context(tc.tile_pool(name="const", bufs=1))
        big = ctx.enter_context(tc.tile_pool(name="big", bufs=1))

        ident16 = const.tile([128, 128], F16)
        make_identity(nc, ident16[:, :])

        w16 = const.tile([128, RC * R], F16)
        nc.sync.dma_start(out=w16[:, :], in_=w_d)
        win16 = const.tile([128, R], F16)
        nc.sync.dma_start(out=win16[:, :], in_=wi_d)

        zbuf = big.tile([128, RC * T * B], F16)
        xT16 = big.tile([128, T * B], F16)
        h16 = big.tile([128, (T + 1) * SUP], F16)
        nc.vector.memset(h16[:, 0:SUP], 0.0)

        zv = zbuf[:, :].rearrange("p (m t b) -> p m t b", t=T, b=B)
        zd_v = z_d.rearrange("p (m t b) -> p m t b", t=T, b=B)

        PIECES = [(0, min(128, T))]
        if T > 128:
            PIECES.append((128, min(448, T)))
        if T > 448:
            PIECES.append((448, T))

        # input DMA per piece (noise+b preformatted on host; x transposed)
        for (t0, t1) in PIECES:
            nc.gpsimd.dma_start(out=zv[:, :, t0:t1, :], in_=zd_v[:, :, t0:t1, :])
            nc.gpsimd.dma_start(out=xT16[:, t0 * B:t1 * B],
                                in_=x_d[:, t0 * B:t1 * B])

        ps_z = ctx.enter_context(tc.tile_pool(name="psz", bufs=2, space="PSUM"))

        def emit_prepass_zmm(p0, p1):
            # psum = x @ w_in + noise (identity-inject), then ACT copies the
            # sum back over zbuf -- keeps the Vector engine free for the
            # recurrence-critical DVE ops.
            for z0 in range(p0, p1, ZB):
                nt = min(ZB, p1 - z0)
                for m in range(RC):
                    zps = ps_z.tile([128, ZB * B], F32, tag="zps")
                    nc.tensor.matmul(
                        zps[:, :nt * B],
                        lhsT=win16[:, m * 128:(m + 1) * 128],
                        rhs=xT16[:, z0 * B:(z0 + nt) * B],
                        start=True, stop=False,
                    )
                    zsl = zv[:, m, z0:z0 + nt, :]
                    nc.tensor.matmul(
                        zps[:, :nt * B].rearrange("p (t b) -> p t b", b=B),
                        lhsT=ident16[:, :],
                        rhs=zsl,
                        start=False, stop=True, skip_group_check=True,
                    )
                    nc.scalar.copy(
                        out=zsl,
                        in_=zps[:, :nt * B].rearrange("p (t b) -> p t b", b=B),
                    )

        # ---- recurrence ----
        with tc.tile_pool(name="psA0", bufs=1, space="PSUM") as ps_a0, \
             tc.tile_pool(name="psA1", bufs=1, space="PSUM") as ps_a1, \
             tc.tile_pool(name="psC0", bufs=1, space="PSUM") as ps_c0, \
             tc.tile_pool(name="psC1", bufs=1, space="PSUM") as ps_c1:
            psAs = [ps_a0.tile([128, 512], F32, name="psa0", tag="psa0"),
                    ps_a1.tile([128, 512], F32, name="psa1", tag="psa1")]
            psCs = [ps_c0.tile([128, 512], F32, name="psc0", tag="psc0"),
                    ps_c1.tile([128, 512], F32, name="psc1", tag="psc1")]
            pvAs = [p[:, :].rearrange("p (m c) -> p m c", c=128) for p in psAs]
            pvCs = [p[:, :].rearrange("p (m c) -> p m c", c=128) for p in psCs]

            emit_prepass_zmm(*PIECES[0])

            # Schedule (steady-state period search, ~789ns model; one DVE
            # read window per psum bank to respect bank-collision rules):
            # bank A holds regions m0 (col 0) / m3 (col 128); bank C holds
            # m1 (col 0) / m2 (col 128). DVE ops: D_C = (1,2), D_A = (0,3).
            ORDER = [(2, 2), (0, 1), (1, 1), (0, 2), (2, 1), (2, 3), (1, 2),
                     (1, 0), (2, 0), (1, 3), (3, 0), (0, 3), (3, 1), (3, 3),
                     (0, 0), (3, 2)]
            LAST = {}
            for i, (m, k) in enumerate(ORDER):
                LAST[m] = i
            BANK = {0: 0, 3: 0, 1: 1, 2: 1}
            COL = {0: 0, 3: 128, 1: 0, 2: 128}
            chain = {"pe": None, "ve": None}

            def _chain(key, ins):
                if chain[key] is not None:
                    tile.add_dep_helper(ins.ins, chain[key].ins, sync=False,
                                        reason="force stream order")
                chain[key] = ins
                return ins

            for t in range(T):
                for pi in range(1, len(PIECES)):
                    if t == PIECES[pi][0] - 64:
                        emit_prepass_zmm(*PIECES[pi])
                par = t % 2
                rd = t * SUP
                wr = (t + 1) * SUP
                banks = [psAs[par], psCs[par]]
                hv_wr = h16[:, wr:wr + SUP].rearrange("p (m c) -> p m c", c=B)
                hv_rd = h16[:, rd:rd + SUP].rearrange("p (m c) -> p m c", c=B)

                def kmm(m, k, stop):
                    ps = banks[BANK[m]]
                    off = COL[m]
                    return _chain("pe", nc.tensor.matmul(
                        ps[:, off:off + B],
                        lhsT=w16[:, k * R + m * 128:k * R + (m + 1) * 128],
                        rhs=h16[:, rd + k * B:rd + (k + 1) * B],
                        start=False, stop=stop, skip_group_check=True,
                    ))

                def imm2(b):
                    pv = (pvAs if b == 0 else pvCs)[par]
                    # bank0 covers m={0,3} (cols 0,128); bank1 covers m={1,2}
                    zsl = zv[:, 0::3, t, :] if b == 0 else zv[:, 1:3, t, :]
                    return _chain("pe", nc.tensor.matmul(
                        pv[:, 0:2, 0:B], lhsT=ident16[:, :],
                        rhs=zsl,
                        start=True, stop=False, skip_group_check=True,
                    ))

                def dve(b):
                    pv = (pvAs if b == 0 else pvCs)[par]
                    hsl_w = hv_wr[:, 0::3, :] if b == 0 else hv_wr[:, 1:3, :]
                    hsl_r = hv_rd[:, 0::3, :] if b == 0 else hv_rd[:, 1:3, :]
                    _chain("ve", nc.vector._custom_dve(
                        LEAKY_RELU_ACC,
                        out=hsl_w,
                        in0=pv[:, 0:2, 0:B],
                        in1=hsl_r,
                        s0=ALPHA, s1=LEAK))

                placed = [False, False]
                done_c = max(LAST[1], LAST[2])
                done_a = max(LAST[0], LAST[3])
                for i, (m, k) in enumerate(ORDER):
                    b = BANK[m]
                    if not placed[b]:
                        imm2(b)
                        placed[b] = True
                    kmm(m, k, stop=(LAST[m] == i))
                    if i == done_c:
                        dve(1)
                    elif i == done_a:
                        dve(0)

                # output drain: raw fp16 slabs, host does the reshape
                if (t + 1) % 128 == 0 or t == T - 1:
                    t0 = (t // 128) * 128
                    nc.sync.dma_start(
                        out=o_d[:, t0 * SUP:(t + 1) * SUP],
                        in_=h16[:, (t0 + 1) * SUP:(t + 2) * SUP],
                    )

    if use_bacc:
        nc.compile()
    return nc


def host_prep(x, w_in, w_rec, b_rec, ei_mask, autapse_mask, noise):
    """Host-side weight prep + layout marshalling + batch shard."""
    ei = np.diagonal(np.asarray(ei_mask)).astype(np.float32)
    w_eff = ei[:, None] * (np.asarray(w_rec) * np.asarray(autapse_mask))
    # w16[p, k*512 + m*128 + c] = w_eff[k*128+p, m*128+c]
    w16 = np.ascontiguousarray(
        w_eff.reshape(RC, 128, RC, 128).transpose(1, 0, 2, 3)
        .reshape(128, RC * R)).astype(np.float16)
    win16 = np.asarray(w_in).astype(np.float16)
    x = np.asarray(x, dtype=np.float32)
    T = x.shape[1]
    nz = (np.asarray(noise, dtype=np.float32)
          + np.asarray(b_rec, dtype=np.float32)).astype(np.float16)
    x16 = x.astype(np.float16)
    bs = x.shape[0] // N_CORES
    in_maps = []
    for c in range(N_CORES):
        xc = x16[c * bs:(c + 1) * bs]                      # (B, T, NIN)
        nc_ = nz[c * bs:(c + 1) * bs]                      # (B, T, R)
        xT = np.ascontiguousarray(
            xc.transpose(2, 1, 0).reshape(128, T * B))     # [i, t*8+b]
        nzc = np.ascontiguousarray(
            nc_.reshape(B, T, RC, 128).transpose(3, 2, 1, 0)
            .reshape(128, RC * T * B))                     # [p, m, t, b]
        in_maps.append({
            "nz16": nzc,
            "xT16": xT,
            "w16": w16,
            "win16": win16,
        })
    return in_maps, w_eff.astype(np.float32)


def reference_np(x, w_in, b_rec, w_eff, noise, T=None):
    x = np.asarray(x, np.float32)
    if T is None:
        T = x.shape[1]
    z = np.einsum("bti,ir->btr", x[:, :T], np.asarray(w_in)) \
        + np.asarray(noise)[:, :T] + np.asarray(b_rec)
    h = np.zeros((x.shape[0], w_eff.shape[0]), np.float32)
    outs = []
    for t in range(T):
        pre = z[:, t] + h @ w_eff
        h = LEAK * h + ALPHA * np.maximum(pre, 0.0)
        outs.append(h.copy())
    return np.stack(outs, axis=1)


# ---------------------------------------------------------------------------
# harness entry point
# ---------------------------------------------------------------------------
_NC_CACHE = {}


def kernel(x, w_in, w_rec, b_rec, ei_mask, autapse_mask, noise):
    from concourse.bass_utils import run_bass_kernel_spmd

    x = np.asarray(x)
    T = x.shape[1]
    in_maps, _ = host_prep(x, w_in, w_rec, b_rec, ei_mask, autapse_mask, noise)
    if T not in _NC_CACHE:
        _NC_CACHE[T] = build_nc(T=T)
    nc = _NC_CACHE[T]
    res = run_bass_kernel_spmd(nc, in_maps, core_ids=list(range(N_CORES)))
    outs = []
    for r in res.results:
        a = r["out16"].reshape(128, T, RC, B)
        outs.append(np.ascontiguousarray(a.transpose(3, 1, 2, 0))
                    .reshape(B, T, R).astype(np.float32))
    return np.concatenate(outs, axis=0)


# revision 10
# speedup vs baseline: 1.4784x; 1.0037x over previous
"""BioRNN Trainium2 kernel.

Per-core math (batch-sharded 8-way across cores, B=8 per core):
    z_t = x_t @ w_in + noise_t + b_rec          (precomputed, fp16, zbuf)
    p_t = z_t + h_{t-1} @ W                     (psum, rebuilt each step)
    h_t = 0.8*h_{t-1} + relu(0.2 * p_t)         (one fused DVE op per bank)

Layouts (partition dim = r-chunk of 128; 4 chunks m=0..3):
  zbuf sbuf fp16 (128, 4*T*8)    col = m*(T*8) + t*8 + b   (m-major planes)
  h16  sbuf fp16 (128, (T+1)*32) col = s*32 + m*8 + b, slot s = h_{s-1}
  w16  sbuf fp16 (128, 4*512)    [p, k*512 + m*128 + c] = W[k*128+p, m*128+c]
  xT16 sbuf fp16 (128, T*8)      col = t*8 + b  (n_in on partitions)

Per step: z is injected into psum by one identity-matmul per bank
(start=True clears the bank), 16 weight matmuls (m,k) accumulate
h_{t-1} @ W, and one fused DVE op per bank computes h_t from {psum,
h_old}. Two psum banks per parity; regions m={0,3} share bank A (cols
0/128), m={1,2} share bank C, so each bank has exactly one DVE read
window per step (psum bank-collision rule). The matmul stream order and
bank pairing come from a steady-state period search over the measured
latency model (27ns issue, 167ns drain, 124ns sem wake, 174ns DVE,
54ns sem); no_sync dep edges pin the searched order.

Output: h16 slabs DMA'd raw to DRAM fp16; host reshapes to (b, t, r) f32.
"""

import numpy as np
from contextlib import ExitStack

import concourse.bass as bass
import concourse.mybir as mybir
import concourse.tile as tile
from concourse import bacc
from concourse import dve_ops
from concourse.dve_spec import Spec, Src0, Src1, C0, C1, relu as _relu, lower
from concourse.dve_uop import DveOpSpec
from concourse.masks import make_identity


def _register_leaky_relu_acc():
    """Register fused out = relu(in0*s0) + in1*s1 custom DVE op (idempotent)."""
    name = "LEAKY_RELU_ACC_BIO"
    for o in dve_ops.OPS:
        if o.name == name:
            return o
    opcode = max(dve_ops._SUB_OPCODE_FOR_NAME.values()) + 1
    assert opcode < 0x20
    dve_ops._SUB_OPCODE_FOR_NAME[name] = opcode

    def _ref(in0, in1, c0, c1, c2):
        a = in0.astype(np.float32).reshape(in0.shape[0], -1)
        b = in1.astype(np.float32).reshape(in1.shape[0], -1)
        s = np.maximum(np.nan_to_num(a * c0, nan=0.0, posinf=np.inf,
                                     neginf=-np.inf), 0) + b * c1
        return s.reshape(in0.shape)

    spec = Spec(body=_relu(Src0 * C0) + Src1 * C1, reference=_ref)
    shas = {}
    for ver in ("v3", "v4"):
        s = DveOpSpec(name=name, opcode=opcode, uops=lower(spec, ver=ver),
                      rd1_en=True)
        shas[ver] = s.sha(ver)
    op = dve_ops.DveOp(name, spec, subdim=False, uops_sha=shas)
    dve_ops.OPS.append(op)
    dve_ops.CUSTOM_DVE_SPECS[name] = spec
    return op


LEAKY_RELU_ACC = _register_leaky_relu_acc()

F32 = mybir.dt.float32
F16 = mybir.dt.float16
AOP = mybir.AluOpType

B = 8            # batch per core
R = 512          # n_rec
NIN = 128        # n_in
RC = 4           # r chunks
SUP = RC * B     # 32 cols per step supertile
N_CORES = 8
ALPHA = 0.2
LEAK = 1.0 - ALPHA


def build_nc(T=1000, use_bacc=True):
    nc = bacc.Bacc() if use_bacc else bass.Bass()

    z_d = nc.dram_tensor("nz16", [128, RC * T * B], F16, kind="ExternalInput").ap()
    x_d = nc.dram_tensor("xT16", [128, T * B], F16, kind="ExternalInput").ap()
    w_d = nc.dram_tensor("w16", [128, RC * R], F16, kind="ExternalInput").ap()
    wi_d = nc.dram_tensor("win16", [NIN, R], F16, kind="ExternalInput").ap()
    o_d = nc.dram_tensor("out16", [128, T * SUP], F16, kind="ExternalOutput").ap()

    ZB = 64  # zmm steps per prepass matmul (64 steps = 512 moving cols)

    with tile.TileContext(nc) as tc, ExitStack() as ctx:
        const = ctx.enter_context(tc.tile_pool(name="const", bufs=1))
        big = ctx.enter_context(tc.tile_pool(name="big", bufs=1))

        ident16 = const.tile([128, 128], F16)
        make_identity(nc, ident16[:, :])

        w16 = const.tile([128, RC * R], F16)
        nc.sync.dma_start(out=w16[:, :], in_=w_d)
        win16 = const.tile([128, R], F16)
        nc.sync.dma_start(out=win16[:, :], in_=wi_d)

        zbuf = big.tile([128, RC * T * B], F16)
        xT16 = big.tile([128, T * B], F16)
        h16 = big.tile([128, (T + 1) * SUP], F16)
        nc.vector.memset(h16[:, 0:SUP], 0.0)

        zv = zbuf[:, :].rearrange("p (m t b) -> p m t b", t=T, b=B)
        zd_v = z_d.rearrange("p (m t b) -> p m t b", t=T, b=B)

        PIECES = [(0, min(128, T))]
        if T > 128:
            PIECES.append((128, min(448, T)))
        if T > 448:
            PIECES.append((448, T))

        # input DMA per piece (noise+b preformatted on host; x transposed)
        for (t0, t1) in PIECES:
            nc.gpsimd.dma_start(out=zv[:, :, t0:t1, :], in_=zd_v[:, :, t0:t1, :])
            nc.gpsimd.dma_start(out=xT16[:, t0 * B:t1 * B],
                                in_=x_d[:, t0 * B:t1 * B])

        ps_z = ctx.enter_context(tc.tile_pool(name="psz", bufs=2, space="PSUM"))

        def emit_prepass_zmm(p0, p1):
            # zbuf += x @ w_in  (noise + b already in zbuf from DMA)
            for z0 in range(p0, p1, ZB):
                nt = min(ZB, p1 - z0)
                for m in range(RC):
                    zps = ps_z.tile([128, ZB * B], F32, tag="zps")
                    nc.tensor.matmul(
                        zps[:, :nt * B],
                        lhsT=win16[:, m * 128:(m + 1) * 128],
                        rhs=xT16[:, z0 * B:(z0 + nt) * B],
                        start=True, stop=True,
                    )
                    zsl = zv[:, m, z0:z0 + nt, :]
                    nc.vector.scalar_tensor_tensor(
                        out=zsl,
                        in0=zps[:, :nt * B].rearrange("p (t b) -> p t b", b=B),
                        scalar=0.0, in1=zsl,
                        op0=AOP.add, op1=AOP.add,
                    )

        # ---- recurrence ----
        with tc.tile_pool(name="psA0", bufs=1, space="PSUM") as ps_a0, \
             tc.tile_pool(name="psA1", bufs=1, space="PSUM") as ps_a1, \
             tc.tile_pool(name="psC0", bufs=1, space="PSUM") as ps_c0, \
             tc.tile_pool(name="psC1", bufs=1, space="PSUM") as ps_c1:
            psAs = [ps_a0.tile([128, 512], F32, name="psa0", tag="psa0"),
                    ps_a1.tile([128, 512], F32, name="psa1", tag="psa1")]
            psCs = [ps_c0.tile([128, 512], F32, name="psc0", tag="psc0"),
                    ps_c1.tile([128, 512], F32, name="psc1", tag="psc1")]
            pvAs = [p[:, :].rearrange("p (m c) -> p m c", c=128) for p in psAs]
            pvCs = [p[:, :].rearrange("p (m c) -> p m c", c=128) for p in psCs]

            emit_prepass_zmm(*PIECES[0])

            # Schedule (steady-state period search, ~789ns model; one DVE
            # read window per psum bank to respect bank-collision rules):
            # bank A holds regions m0 (col 0) / m3 (col 128); bank C holds
            # m1 (col 0) / m2 (col 128). DVE ops: D_C = (1,2), D_A = (0,3).
            ORDER = [(2, 2), (0, 1), (1, 1), (0, 2), (2, 1), (2, 3), (1, 2),
                     (1, 0), (2, 0), (1, 3), (3, 0), (0, 3), (3, 1), (3, 3),
                     (0, 0), (3, 2)]
            LAST = {}
            for i, (m, k) in enumerate(ORDER):
                LAST[m] = i
            BANK = {0: 0, 3: 0, 1: 1, 2: 1}
            COL = {0: 0, 3: 128, 1: 0, 2: 128}
            chain = {"pe": None, "ve": None}

            def _chain(key, ins):
                if chain[key] is not None:
                    tile.add_dep_helper(ins.ins, chain[key].ins, sync=False,
                                        reason="force stream order")
                chain[key] = ins
                return ins

            for t in range(T):
                for pi in range(1, len(PIECES)):
                    if t == PIECES[pi][0] - 64:
                        emit_prepass_zmm(*PIECES[pi])
                par = t % 2
                rd = t * SUP
                wr = (t + 1) * SUP
                banks = [psAs[par], psCs[par]]
                hv_wr = h16[:, wr:wr + SUP].rearrange("p (m c) -> p m c", c=B)
                hv_rd = h16[:, rd:rd + SUP].rearrange("p (m c) -> p m c", c=B)

                def kmm(m, k, stop):
                    ps = banks[BANK[m]]
                    off = COL[m]
                    return _chain("pe", nc.tensor.matmul(
                        ps[:, off:off + B],
                        lhsT=w16[:, k * R + m * 128:k * R + (m + 1) * 128],
                        rhs=h16[:, rd + k * B:rd + (k + 1) * B],
                        start=False, stop=stop, skip_group_check=True,
                    ))

                def imm2(b):
                    pv = (pvAs if b == 0 else pvCs)[par]
                    # bank0 covers m={0,3} (cols 0,128); bank1 covers m={1,2}
                    zsl = zv[:, 0::3, t, :] if b == 0 else zv[:, 1:3, t, :]
                    return _chain("pe", nc.tensor.matmul(
                        pv[:, 0:2, 0:B], lhsT=ident16[:, :],
                        rhs=zsl,
                        start=True, stop=False, skip_group_check=True,
                    ))

                def dve(b):
                    pv = (pvAs if b == 0 else pvCs)[par]
                    hsl_w = hv_wr[:, 0::3, :] if b == 0 else hv_wr[:, 1:3, :]
                    hsl_r = hv_rd[:, 0::3, :] if b == 0 else hv_rd[:, 1:3, :]
                    _chain("ve", nc.vector._custom_dve(
                        LEAKY_RELU_ACC,
                        out=hsl_w,
                        in0=pv[:, 0:2, 0:B],
                        in1=hsl_r,
                        s0=ALPHA, s1=LEAK))

                placed = [False, False]
                done_c = max(LAST[1], LAST[2])
                done_a = max(LAST[0], LAST[3])
                for i, (m, k) in enumerate(ORDER):
                    b = BANK[m]
                    if not placed[b]:
                        imm2(b)
                        placed[b] = True
                    kmm(m, k, stop=(LAST[m] == i))
                    if i == done_c:
                        dve(1)
                    elif i == done_a:
                        dve(0)

                # output drain: raw fp16 slabs, host does the reshape
                if (t + 1) % 128 == 0 or t == T - 1:
                    t0 = (t // 128) * 128
                    nc.sync.dma_start(
                        out=o_d[:, t0 * SUP:(t + 1) * SUP],
                        in_=h16[:, (t0 + 1) * SUP:(t + 2) * SUP],
                    )

    if use_bacc:
        nc.compile()
    return nc


def host_prep(x, w_in, w_rec, b_rec, ei_mask, autapse_mask, noise):
    """Host-side weight prep + layout marshalling + batch shard."""
    ei = np.diagonal(np.asarray(ei_mask)).astype(np.float32)
    w_eff = ei[:, None] * (np.asarray(w_rec) * np.asarray(autapse_mask))
    # w16[p, k*512 + m*128 + c] = w_eff[k*128+p, m*128+c]
    w16 = np.ascontiguousarray(
        w_eff.reshape(RC, 128, RC, 128).transpose(1, 0, 2, 3)
        .reshape(128, RC * R)).astype(np.float16)
    win16 = np.asarray(w_in).astype(np.float16)
    x = np.asarray(x, dtype=np.float32)
    T = x.shape[1]
    nz = (np.asarray(noise, dtype=np.float32)
          + np.asarray(b_rec, dtype=np.float32)).astype(np.float16)
    x16 = x.astype(np.float16)
    bs = x.shape[0] // N_CORES
    in_maps = []
    for c in range(N_CORES):
        xc = x16[c * bs:(c + 1) * bs]                      # (B, T, NIN)
        nc_ = nz[c * bs:(c + 1) * bs]                      # (B, T, R)
        xT = np.ascontiguousarray(
            xc.transpose(2, 1, 0).reshape(128, T * B))     # [i, t*8+b]
        nzc = np.ascontiguousarray(
            nc_.reshape(B, T, RC, 128).transpose(3, 2, 1, 0)
            .reshape(128, RC * T * B))                     # [p, m, t, b]
        in_maps.append({
            "nz16": nzc,
            "xT16": xT,
            "w16": w16,
            "win16": win16,
        })
    return in_maps, w_eff.astype(np.float32)


def reference_np(x, w_in, b_rec, w_eff, noise, T=None):
    x = np.asarray(x, np.float32)
    if T is None:
        T = x.shape[1]
    z = np.einsum("bti,ir->btr", x[:, :T], np.asarray(w_in)) \
        + np.asarray(noise)[:, :T] + np.asarray(b_rec)
    h = np.zeros((x.shape[0], w_eff.shape[0]), np.float32)
    outs = []
    for t in range(T):
        pre = z[:, t] + h @ w_eff
        h = LEAK * h + ALPHA * np.maximum(pre, 0.0)
        outs.append(h.copy())
    return np.stack(outs, axis=1)


# ---------------------------------------------------------------------------
# harness entry point
# ---------------------------------------------------------------------------
_NC_CACHE = {}


def kernel(x, w_in, w_rec, b_rec, ei_mask, autapse_mask, noise):
    from concourse.bass_utils import run_bass_kernel_spmd

    x = np.asarray(x)
    T = x.shape[1]
    in_maps, _ = host_prep(x, w_in, w_rec, b_rec, ei_mask, autapse_mask, noise)
    if T not in _NC_CACHE:
        _NC_CACHE[T] = build_nc(T=T)
    nc = _NC_CACHE[T]
    res = run_bass_kernel_spmd(nc, in_maps, core_ids=list(range(N_CORES)))
    outs = []
    for r in res.results:
        a = r["out16"].reshape(128, T, RC, B)
        outs.append(np.ascontiguousarray(a.transpose(3, 1, 2, 0))
                    .reshape(B, T, R).astype(np.float32))
    return np.concatenate(outs, axis=0)
